# revision 25
# baseline (speedup 1.0000x reference)
"""CVKAN layer Trainium2 kernel.

Math (per reference):
    basis[b, i, k] = exp(-((x_part[b,i] - grid[k%8]) / h)^2), part = re if k<8 else im
    out_re[b, o]   = sum_{i,k} basis[b,i,k] * coeffs_re[i,o,k] + bias_re[o]
    out_im[b, o]   = sum_{i,k} basis[b,i,k] * coeffs_im[i,o,k] + bias_im[o]
    out = out_re + 1j*out_im   (complex64)

Device strategy (pure data-parallel over batch, 8 cores, no collectives):
  - Load x tiles [128b, 128(i_re|i_im)], PE-transpose each 128x128 block so
    the contraction index (part, i) sits on partitions: T [128, b].
  - Grid points j=0..2 evaluated directly on ScalarE via
    Derivative_Erf(z) = 2/sqrt(pi) * exp(-z^2) (one pass per grid point,
    prefactor folded into the weights). Grid points 3..7 via the exact
    multiplicative recurrence b_{j+1} = (b_j * c) * V with
    V = exp(2*z_2 - 1), split across VectorE and GpSimd. Two chunks carry
    surplus e^2/e^6 factors absorbed by the weights.
  - TensorE accumulates out^T[32, b] += W_j^T @ basis_j (weights stationary,
    float32r single-pass streaming), 8 chunks per PSUM accumulation group.
  - Tiles are processed in software-pipelined pairs so the serial recurrence
    of one tile overlaps the ScalarE work of the next; ACT ops are grouped
    per pair to minimize activation-table switches (derf and exp live in
    different table sets).
  - out^T [32, 8192] fp32 per core is stored; the host interleaves to
    complex64 during gather/unshard.
"""

import sys

import numpy as np

if "/opt/trn_rl_repo" not in sys.path:
    sys.path.append("/opt/trn_rl_repo")

B = 65536
IN = 64
OUT = 16
NB = 8
N_CORES = 8
B_CORE = B // N_CORES  # 8192
H = 2.0 / (NB - 1)
GRID = [-1.0 + j * H for j in range(NB)]
# Surplus exponents carried by chain-produced chunks of cs=2 tiles
# (folded into W rows 4 and 6; rows 8/9 hold unscaled copies for the tail).
SIGMA = {4: 2.0, 6: 6.0}

TILE_SIZES = [1024] * 8
CHAIN_STARTS = [2, 2, 2, 2, 2, 2, 2, 5]
PAIRS = [(0, 1, 2, 3), (4, 5, 6, 7)]
assert sum(TILE_SIZES) == B_CORE

BASIS_MODE = "derf"

_CACHE = {}


def _build_module():
    import concourse.mybir as mybir
    import concourse.tile as tile
    from concourse import bacc
    from concourse.masks import make_identity

    f32 = mybir.dt.float32
    f32r = mybir.dt.float32r
    nc = bacc.Bacc("TRN2", target_bir_lowering=False, debug=False,
                   num_devices=N_CORES)

    x_re = nc.dram_tensor("x_re", [B_CORE, IN], f32, kind="ExternalInput")
    x_im = nc.dram_tensor("x_im", [B_CORE, IN], f32, kind="ExternalInput")
    w = nc.dram_tensor("w", [NB + 2, 128, 2 * OUT], f32r, kind="ExternalInput")
    bias32 = nc.dram_tensor("bias32", [1, 2 * OUT], f32, kind="ExternalInput")
    out_t = nc.dram_tensor("out_t", [2 * OUT, B_CORE], f32,
                           kind="ExternalOutput")

    Exp = mybir.ActivationFunctionType.Exp
    DErf = mybir.ActivationFunctionType.Derivative_Erf

    with tile.TileContext(nc) as tc:
        with (
            tc.tile_pool(name="consts", bufs=1) as consts,
            tc.tile_pool(name="xin", bufs=6) as xpool,
            tc.tile_pool(name="tpsum", bufs=1, space="PSUM") as tpsum,
            tc.tile_pool(name="tsb", bufs=4) as tpool,
            tc.tile_pool(name="basis", bufs=20) as bpool,
            tc.tile_pool(name="vmul", bufs=5) as vpool,
            tc.tile_pool(name="opsum", bufs=3, space="PSUM") as opsum,
            tc.tile_pool(name="osb", bufs=2) as opool,
        ):
            identity = consts.tile([128, 128], f32)
            make_identity(nc, identity)
            w_sb = consts.tile([128, (NB + 2) * 2 * OUT], f32r)
            nc.sync.dma_start(
                out=w_sb[:].rearrange("p (j o) -> p j o", j=NB + 2),
                in_=w.ap().rearrange("j p o -> p j o"),
            )
            bias_sb = consts.tile([2 * OUT, 1], f32)
            nc.sync.dma_start(out=bias_sb[:],
                              in_=bias32.ap().rearrange("a o -> o a"))
            # Activation bias columns: col j = -grid[j]/h (Gaussian centers),
            # col NB+s = -2*grid[s]/h - 1 (chain multiplier V_s).
            gbias = consts.tile([128, 2 * NB], f32)
            for j in range(NB):
                nc.vector.memset(gbias[:, j:j + 1], -GRID[j] / H)
                nc.vector.memset(gbias[:, NB + j:NB + j + 1],
                                 -2.0 * GRID[j] / H - 1.0)

            class Job:
                def __init__(self, idx):
                    self.bt = TILE_SIZES[idx]
                    self.cs = CHAIN_STARTS[idx]
                    self.base = sum(TILE_SIZES[:idx])
                    if self.cs == 2:
                        self.sigma, self.wmap = SIGMA, {}
                        self.pool_steps = (1, 3)
                    else:
                        self.sigma = {}
                        self.wmap = {4: NB, 6: NB + 1}
                        self.pool_steps = ()
                    self.T = None
                    self.out_ps = None
                    self.prev = None
                    self.s_in = 0.0

            jobs = [Job(i) for i in range(len(TILE_SIZES))]

            def build_T(job):
                job.T = tpool.tile([128, job.bt], f32, tag="T")
                for q in range(job.bt // 512):
                    xcat = xpool.tile([128, 512], f32)
                    xv = xcat[:].rearrange("p (nb c) -> p nb c", c=128)
                    b0 = job.base + q * 512
                    nc.sync.dma_start(
                        out=xv[:, :, 0:IN],
                        in_=x_re.ap()[b0:b0 + 512, :]
                            .rearrange("(nb p) i -> p nb i", p=128),
                    )
                    nc.sync.dma_start(
                        out=xv[:, :, IN:128],
                        in_=x_im.ap()[b0:b0 + 512, :]
                            .rearrange("(nb p) i -> p nb i", p=128),
                    )
                    tp = tpsum.tile([128, 512], f32)
                    for r in range(4):
                        nc.tensor.transpose(
                            tp[:, r * 128:(r + 1) * 128],
                            xcat[:, r * 128:(r + 1) * 128],
                            identity,
                        )
                    nc.vector.tensor_copy(job.T[:, q * 512:(q + 1) * 512],
                                          tp[:])

            def mms(job, j, basis, start, stop):
                wj = job.wmap.get(j, j)
                for s in range(job.bt // 512):
                    nc.tensor.matmul(
                        job.out_ps[:, s * 512:(s + 1) * 512],
                        w_sb[:, wj * 2 * OUT:(wj + 1) * 2 * OUT],
                        basis[:, s * 512:(s + 1) * 512],
                        start=start,
                        stop=stop,
                    )

            def direct_phase(job):
                job.out_ps = opsum.tile([2 * OUT, job.bt], f32, tag="out_ps")
                for j in range(job.cs + 1):
                    basis = bpool.tile([128, job.bt], f32r, tag="basis")
                    nc.scalar.activation(basis[:], job.T[:], DErf,
                                         bias=gbias[:, j:j + 1],
                                         scale=1.0 / H)
                    mms(job, j, basis, j == 0, False)
                    job.prev = basis

            def vexp_phase(job):
                job.vmul = vpool.tile([128, job.bt], f32, tag="vmul")
                nc.scalar.activation(job.vmul[:], job.T[:], Exp,
                                     bias=gbias[:, NB + job.cs:NB + job.cs + 1],
                                     scale=2.0 / H)

            def chain_step(job, m):
                j = job.cs + 1 + m
                if j >= NB:
                    return
                target = job.sigma.get(j, 0.0)
                cexp = target - job.s_in - 2.0 * m
                basis = bpool.tile([128, job.bt], f32r, tag="basis")
                if m in job.pool_steps:
                    assert abs(cexp) < 1e-9, cexp
                    nc.gpsimd.tensor_tensor(
                        out=basis[:], in0=job.prev[:].bitcast(f32),
                        in1=job.vmul[:], op=mybir.AluOpType.mult,
                    )
                else:
                    nc.vector.scalar_tensor_tensor(
                        out=basis[:],
                        in0=job.prev[:].bitcast(f32),
                        scalar=float(np.exp(cexp)),
                        in1=job.vmul[:],
                        op0=mybir.AluOpType.mult,
                        op1=mybir.AluOpType.mult,
                    )
                mms(job, j, basis, False, j == NB - 1)
                job.prev = basis
                job.s_in = target

            def store_phase(job):
                out_sb = opool.tile([2 * OUT, job.bt], f32, tag="out_sb")
                nc.vector.tensor_scalar_add(out_sb[:], job.out_ps[:],
                                            bias_sb[:])
                nc.sync.dma_start(
                    out=out_t.ap()[:, job.base:job.base + job.bt],
                    in_=out_sb[:],
                )

            for t in PAIRS[0]:
                build_T(jobs[t])
            for pi, pair in enumerate(PAIRS):
                for t in pair:
                    direct_phase(jobs[t])
                for t in pair:
                    vexp_phase(jobs[t])
                if pi + 1 < len(PAIRS):
                    for t in PAIRS[pi + 1]:
                        build_T(jobs[t])
                for m in range(NB):
                    for t in pair:
                        chain_step(jobs[t], m)
                for t in pair:
                    store_phase(jobs[t])

    nc.compile()
    return nc


def _get_module():
    if "nc" not in _CACHE:
        _CACHE["nc"] = _build_module()
    return _CACHE["nc"]


def _build_w(coeffs_re, coeffs_im):
    w = np.empty((NB + 2, 128, 2 * OUT), dtype=np.float32)
    w[:NB, :IN, :OUT] = np.transpose(coeffs_re[:, :, :NB], (2, 0, 1))
    w[:NB, :IN, OUT:] = np.transpose(coeffs_im[:, :, :NB], (2, 0, 1))
    w[:NB, IN:, :OUT] = np.transpose(coeffs_re[:, :, NB:], (2, 0, 1))
    w[:NB, IN:, OUT:] = np.transpose(coeffs_im[:, :, NB:], (2, 0, 1))
    if BASIS_MODE == "derf":
        w[:NB] *= np.float32(np.sqrt(np.pi) / 2.0)
    # Rows 8/9: unscaled copies of chunks 4/6 for the tail tile; rows 4/6
    # absorb the chain surpluses of the main tiles.
    w[NB] = w[4]
    w[NB + 1] = w[6]
    for j, sig in SIGMA.items():
        w[j] *= np.float32(np.exp(-sig))
    return w


def kernel(x_re, x_im, coeffs_re, coeffs_im, bias_re, bias_im):
    from concourse.bass_utils import run_bass_kernel_spmd

    nc = _get_module()
    w = _build_w(np.asarray(coeffs_re), np.asarray(coeffs_im))
    bias32 = np.concatenate(
        [np.asarray(bias_re), np.asarray(bias_im)]
    ).astype(np.float32).reshape(1, 2 * OUT)

    x_re = np.ascontiguousarray(x_re, dtype=np.float32)
    x_im = np.ascontiguousarray(x_im, dtype=np.float32)
    in_maps = [
        {
            "x_re": x_re[c * B_CORE:(c + 1) * B_CORE],
            "x_im": x_im[c * B_CORE:(c + 1) * B_CORE],
            "w": w,
            "bias32": bias32,
        }
        for c in range(N_CORES)
    ]
    res = run_bass_kernel_spmd(nc, in_maps, core_ids=list(range(N_CORES)))
    out = np.empty((B, OUT), dtype=np.complex64)
    for c in range(N_CORES):
        ot = res.results[c]["out_t"]  # [32, B_CORE] fp32
        out[c * B_CORE:(c + 1) * B_CORE] = (ot[:OUT].T + 1j * ot[OUT:].T)
    return out


# revision 26
# speedup vs baseline: 1.1234x; 1.1234x over previous
"""CVKAN layer Trainium2 kernel.

Math (per reference):
    basis[b, i, k] = exp(-((x_part[b,i] - grid[k%8]) / h)^2), part = re if k<8 else im
    out_re[b, o]   = sum_{i,k} basis[b,i,k] * coeffs_re[i,o,k] + bias_re[o]
    out_im[b, o]   = sum_{i,k} basis[b,i,k] * coeffs_im[i,o,k] + bias_im[o]
    out = out_re + 1j*out_im   (complex64)

Device strategy (pure data-parallel over batch across 8 cores, no
collectives needed):
  - Load x tiles [128b, 128(i_re|i_im)] and PE-transpose each 128x128 block
    so the contraction index (part, i) sits on partitions: T [128, b].
  - For each grid point j (8 per part): one contraction chunk.
    basis_j = (2/sqrt(pi))*exp(-z^2), z = (T - g_j)/h, evaluated in a
    single ScalarE pass via Derivative_Erf (the 2/sqrt(pi) prefactor is
    folded into the weights host-side). The grid shift is the activation's
    free affine bias, so each chunk differs only in a per-partition bias
    column.
  - TensorE accumulates out^T[32, b] += W_j^T @ basis_j with the small
    weight matrix stationary and basis streaming as float32r (single-pass
    fp32 streaming, 4x faster than exact fp32 matmul; ~2e-4 output rel err).
  - The complex bias is added during the PSUM->SBUF eviction as a
    per-partition tensor_scalar add on VectorE.
  - out^T [32, 8192] fp32 per core is stored contiguously; the host
    interleaves re/im into complex64 while gathering the batch shards.
  - Tile sizes are graduated (small first tile so ScalarE starts early,
    small last tile so the matmul/store tail after the final activation is
    short). ScalarE is the bottleneck engine (~63us busy of ~77us total);
    TensorE/VectorE/DMA run underneath it.
"""

import sys

import numpy as np

if "/opt/trn_rl_repo" not in sys.path:
    sys.path.append("/opt/trn_rl_repo")

B = 65536
IN = 64
OUT = 16
NB = 8
N_CORES = 8
B_CORE = B // N_CORES  # 8192
H = 2.0 / (NB - 1)
GRID = [-1.0 + j * H for j in range(NB)]

# Graduated tile sizes: small first tile starts ScalarE sooner; small last
# tile shortens the matmul/copy/store tail after the final activation.
TILE_SIZES = [1024, 2048, 2048, 2048, 1024]
assert sum(TILE_SIZES) == B_CORE

_CACHE = {}


def _build_module():
    import concourse.mybir as mybir
    import concourse.tile as tile
    from concourse import bacc
    from concourse.masks import make_identity

    f32 = mybir.dt.float32
    f32r = mybir.dt.float32r
    nc = bacc.Bacc("TRN2", target_bir_lowering=False, debug=False,
                   num_devices=N_CORES)

    x_re = nc.dram_tensor("x_re", [B_CORE, IN], f32, kind="ExternalInput")
    x_im = nc.dram_tensor("x_im", [B_CORE, IN], f32, kind="ExternalInput")
    w = nc.dram_tensor("w", [NB, 128, 2 * OUT], f32r, kind="ExternalInput")
    bias32 = nc.dram_tensor("bias32", [1, 2 * OUT], f32, kind="ExternalInput")
    out_t = nc.dram_tensor("out_t", [2 * OUT, B_CORE], f32,
                           kind="ExternalOutput")

    DErf = mybir.ActivationFunctionType.Derivative_Erf

    with tile.TileContext(nc) as tc:
        with (
            tc.tile_pool(name="consts", bufs=1) as consts,
            tc.tile_pool(name="xin", bufs=4) as xpool,
            tc.tile_pool(name="tpsum", bufs=3, space="PSUM") as tpsum,
            tc.tile_pool(name="tsb", bufs=3) as tpool,
            tc.tile_pool(name="basis", bufs=4) as bpool,
            tc.tile_pool(name="opsum", bufs=1, space="PSUM") as opsum,
            tc.tile_pool(name="osb", bufs=2) as opool,
        ):
            identity = consts.tile([128, 128], f32)
            make_identity(nc, identity)
            w_sb = consts.tile([128, NB * 2 * OUT], f32r)
            nc.sync.dma_start(
                out=w_sb[:].rearrange("p (j o) -> p j o", j=NB),
                in_=w.ap().rearrange("j p o -> p j o"),
            )
            bias_sb = consts.tile([2 * OUT, 1], f32)
            nc.sync.dma_start(out=bias_sb[:],
                              in_=bias32.ap().rearrange("a o -> o a"))
            # Per-chunk activation bias columns: bias_j = -grid[j]/h.
            gbias = consts.tile([128, NB], f32)
            for j in range(NB):
                nc.vector.memset(gbias[:, j:j + 1], -GRID[j] / H)

            base = 0
            for g, bt in enumerate(TILE_SIZES):
                T = tpool.tile([128, bt], f32, tag="T")
                for q in range(bt // 512):
                    # Load 4 b-blocks (512 batch rows) of x_re|x_im columns.
                    xcat = xpool.tile([128, 512], f32)
                    xv = xcat[:].rearrange("p (nb c) -> p nb c", c=128)
                    b0 = base + q * 512
                    nc.sync.dma_start(
                        out=xv[:, :, 0:IN],
                        in_=x_re.ap()[b0:b0 + 512, :]
                            .rearrange("(nb p) i -> p nb i", p=128),
                    )
                    nc.sync.dma_start(
                        out=xv[:, :, IN:128],
                        in_=x_im.ap()[b0:b0 + 512, :]
                            .rearrange("(nb p) i -> p nb i", p=128),
                    )
                    tp = tpsum.tile([128, 512], f32)
                    for r in range(4):
                        nc.tensor.transpose(
                            tp[:, r * 128:(r + 1) * 128],
                            xcat[:, r * 128:(r + 1) * 128],
                            identity,
                        )
                    nc.vector.tensor_copy(T[:, q * 512:(q + 1) * 512], tp[:])

                out_ps = opsum.tile([2 * OUT, bt], f32, tag="out_ps")
                for j in range(NB):
                    basis = bpool.tile([128, bt], f32r, tag="basis")
                    nc.scalar.activation(basis[:], T[:], DErf,
                                         bias=gbias[:, j:j + 1],
                                         scale=1.0 / H)
                    for s in range(bt // 512):
                        nc.tensor.matmul(
                            out_ps[:, s * 512:(s + 1) * 512],
                            w_sb[:, j * 2 * OUT:(j + 1) * 2 * OUT],
                            basis[:, s * 512:(s + 1) * 512],
                            start=(j == 0),
                            stop=(j == NB - 1),
                        )
                out_sb = opool.tile([2 * OUT, bt], f32, tag="out_sb")
                for u in range(bt // 1024):
                    sl = slice(u * 1024, (u + 1) * 1024)
                    nc.vector.tensor_scalar_add(out_sb[:, sl], out_ps[:, sl],
                                                bias_sb[:])
                    nc.sync.dma_start(
                        out=out_t.ap()[:, base + u * 1024:
                                       base + (u + 1) * 1024],
                        in_=out_sb[:, sl],
                    )
                base += bt

    nc.compile()
    return nc


def _get_module():
    if "nc" not in _CACHE:
        _CACHE["nc"] = _build_module()
    return _CACHE["nc"]


def _build_w(coeffs_re, coeffs_im):
    w = np.empty((NB, 128, 2 * OUT), dtype=np.float32)
    w[:, :IN, :OUT] = np.transpose(coeffs_re[:, :, :NB], (2, 0, 1))
    w[:, :IN, OUT:] = np.transpose(coeffs_im[:, :, :NB], (2, 0, 1))
    w[:, IN:, :OUT] = np.transpose(coeffs_re[:, :, NB:], (2, 0, 1))
    w[:, IN:, OUT:] = np.transpose(coeffs_im[:, :, NB:], (2, 0, 1))
    # Fold the Derivative_Erf prefactor 2/sqrt(pi) into the weights.
    w *= np.float32(np.sqrt(np.pi) / 2.0)
    return w


def kernel(x_re, x_im, coeffs_re, coeffs_im, bias_re, bias_im):
    from concourse.bass_utils import run_bass_kernel_spmd

    nc = _get_module()
    w = _build_w(np.asarray(coeffs_re), np.asarray(coeffs_im))
    bias32 = np.concatenate(
        [np.asarray(bias_re), np.asarray(bias_im)]
    ).astype(np.float32).reshape(1, 2 * OUT)

    x_re = np.ascontiguousarray(x_re, dtype=np.float32)
    x_im = np.ascontiguousarray(x_im, dtype=np.float32)
    in_maps = [
        {
            "x_re": x_re[c * B_CORE:(c + 1) * B_CORE],
            "x_im": x_im[c * B_CORE:(c + 1) * B_CORE],
            "w": w,
            "bias32": bias32,
        }
        for c in range(N_CORES)
    ]
    res = run_bass_kernel_spmd(nc, in_maps, core_ids=list(range(N_CORES)))
    out = np.empty((B, OUT), dtype=np.complex64)
    for c in range(N_CORES):
        ot = res.results[c]["out_t"]  # [32, B_CORE] fp32
        out[c * B_CORE:(c + 1) * B_CORE] = (ot[:OUT].T + 1j * ot[OUT:].T)
    return out
